# revision 1
# baseline (speedup 1.0000x reference)
"""CardEncoder Trainium2 kernel.

Model (per sequence of L=16 tokens): embed(32) -> bidirectional LSTM(32) ->
concat final states -> per-batch dense (4096 -> 64) -> tanh.

Strategy (pure data parallel, 8 cores, batch-sharded):
  * Host packs an augmented gather table [10112, 128] bf16 per vocab row:
      [ h_fw slot (zeros 0:32) | embedding (32:64) | 1.0 bias (64) |
        h_bw slot (zeros 65:97) | zeros ]
  * Device gathers rows with dma_gather(transpose=True) -> feature-major
    tiles G[128, T*NT] (columns = (t, seq)); the constant-1 row turns the
    LSTM bias into a matmul row; the zero h-slots are overwritten in-place
    with the running hidden state so each step is ONE K=65 matmul per gate.
  * 4 streams (2 seq-tiles x {fw, bw}) stacked on PSUM partition quarters via
    tile_position col-groups, so all elementwise work runs on full
    [128, NT] tiles.
  * LSTM cell: i,f,o = sigmoid, g = tanh (ACT); c = f*c + i*g, h = o*tanh(c)
    (DVE); h copied into the next step's h-slot of G.
  * Dense head on device; host transposes [64, B] -> [B, 64].
  * mask_zero=True handling: token==0 steps must leave (h, c) unchanged.
    Zero tokens occur w.p. 1e-4; the device ignores masking and the host
    recomputes the ~0.02% of affected sequences exactly and patches the
    affected output rows.
"""

import os
import numpy as np
import ml_dtypes

os.environ.setdefault("JAX_PLATFORMS", "cpu")

import concourse.bass as bass
import concourse.bacc as bacc
import concourse.mybir as mybir
import concourse.tile as tile
from concourse import bass_utils

BF16 = ml_dtypes.bfloat16

B, P, L = 2048, 64, 16
H = 32                      # LSTM units per direction
VOC = 10000
VOCP = 10112                # padded to 79 * 128
N_CORES = 8
B_LOC = B // N_CORES        # 256 batches per core
NSEQ = B_LOC * P            # 16384 sequences per core
NT = 512                    # sequences per tile
T = L

# G tile partition layout (SBUF APs must start at partition 0/32/64/96):
#   [ h_fw slot 0:32 | emb 32:64 | emb copy 64:96 | h_bw slot 96:128 ]
# fw rhs = rows 0:64 [h_fw, emb];  bw rhs = rows 64:128 [emb, h_bw].
HFW0 = 0
EMB0 = 32
EMB1 = 64
HBW0 = 96
KDIM = 64                   # matmul contraction size


def _f32(x):
    return np.asarray(x, np.float32)


# ---------------------------------------------------------------------------
# device kernel
# ---------------------------------------------------------------------------

def build_kernel(nseq=NSEQ, mode="full"):
    ntiles = nseq // NT
    npairs = ntiles // 2
    nbatch = nseq // P

    nc = bacc.Bacc("TRN2", target_bir_lowering=False, debug=False,
                   enable_asserts=False, num_devices=N_CORES)

    table_d = nc.dram_tensor("table", [VOCP, 128], mybir.dt.bfloat16,
                             kind="ExternalInput")
    idx_d = nc.dram_tensor("idx", [128, ntiles * NT * T // 16], mybir.dt.int16,
                           kind="ExternalInput")
    wf_d = nc.dram_tensor("wf", [KDIM, 128], mybir.dt.bfloat16,
                          kind="ExternalInput")
    wb_d = nc.dram_tensor("wb", [128, 128], mybir.dt.bfloat16,
                          kind="ExternalInput")
    bv_d = nc.dram_tensor("bv", [128, 4], mybir.dt.float32,
                          kind="ExternalInput")
    wd_d = nc.dram_tensor("wd", [64, 4096], mybir.dt.bfloat16,
                          kind="ExternalInput")
    bd_d = nc.dram_tensor("bd", [64, 1], mybir.dt.float32,
                          kind="ExternalInput")
    out_d = nc.dram_tensor("out", [64, nbatch], mybir.dt.float32,
                           kind="ExternalOutput")
    state_d = nc.dram_tensor("state", [64, nseq], mybir.dt.bfloat16,
                             kind="ExternalOutput")

    FP32 = mybir.dt.float32
    BF = mybir.dt.bfloat16
    SIG = mybir.ActivationFunctionType.Sigmoid
    TANH = mybir.ActivationFunctionType.Tanh

    with tile.TileContext(nc) as tc:
        with tc.tile_pool(name="const", bufs=1) as cpool:
          with tc.tile_pool(name="gbuf", bufs=2) as gpool, \
               tc.tile_pool(name="work", bufs=2) as wpool, \
               tc.tile_pool(name="zps", bufs=2, space="PSUM") as zpool:

            wf = cpool.tile([KDIM, 128], BF)
            nc.sync.dma_start(out=wf[:, :], in_=wf_d.ap())
            # bw weights live at partition base 64: walrus requires matmul
            # fmap and weight to share the same SB start partition, and the
            # bw rhs is G[64:128]. Host pads to [128, 128] (top half zeros)
            # so the DMA itself writes at partition base 0.
            wb_t = cpool.tile([128, 128], BF)
            nc.sync.dma_start(out=wb_t[:, :], in_=wb_d.ap())
            wb = wb_t[64:128, :]
            bv = cpool.tile([128, 4], FP32)
            nc.sync.dma_start(out=bv[:, :], in_=bv_d.ap())
            idx_sb = cpool.tile([128, ntiles * NT * T // 16], mybir.dt.int16)
            nc.sync.dma_start(out=idx_sb[:, :], in_=idx_d.ap())
            state = cpool.tile([64, nseq], BF)

            IDXW = NT * T // 16      # idx columns per tile

            def gather_tile(j):
                g = gpool.tile([128, 1, T * NT], BF, tag=f"g{j % 2}")
                nc.gpsimd.dma_gather(
                    out_ap=g[:, :, :],
                    in_ap=table_d.ap(),
                    idxs_ap=idx_sb[:, j * IDXW:(j + 1) * IDXW],
                    num_idxs=T * NT,
                    num_idxs_reg=T * NT,
                    elem_size=128,
                    transpose=True,
                    single_packet=False,
                )
                return g

            for pair in range(npairs):
                if mode == "empty":
                    break
                if mode == "compute":
                    ga = gpool.tile([128, 1, T * NT], BF, tag="g0",
                                    name=f"ga{pair}")
                    gb = gpool.tile([128, 1, T * NT], BF, tag="g1",
                                    name=f"gb{pair}")
                else:
                    ga = gather_tile(2 * pair)
                    gb = gather_tile(2 * pair + 1)
                gs = [ga, gb]
                if mode == "gather":
                    # consume G so nothing gets dead-code-eliminated
                    for gi2, g_t in enumerate(gs):
                        col0 = (2 * pair + gi2) * NT
                        nc.vector.tensor_copy(
                            state[0:32, col0:col0 + NT],
                            g_t[32:64, 0, (T - 1) * NT:T * NT])
                    continue
                c_all = wpool.tile([128, NT], FP32, tag="c")

                for tau in range(T):
                    # one PSUM bank per gate, stream s on partition quarter s
                    zt = [zpool.tile([128, NT], FP32, tag=f"z{gi}",
                                     name=f"z{gi}_{pair}_{tau}")
                          for gi in range(4)]
                    for s in range(4):
                        g_t = gs[s // 2]
                        bw = s % 2
                        blk = (T - 1 - tau) if bw else tau
                        lo = EMB1 if bw else HFW0
                        w_s = wb if bw else wf
                        rhs = g_t[lo:lo + KDIM, 0, blk * NT:(blk + 1) * NT]
                        for gi in range(4):   # gate order i,f,g,o
                            nc.tensor.matmul(
                                zt[gi][32 * s:32 * s + 32, :],
                                w_s[:, 32 * gi:32 * gi + 32], rhs,
                                start=True, stop=True,
                                tile_position=(64 if bw else 0, 32 * s))

                    ui = wpool.tile([128, NT], BF, tag="ui")
                    nc.scalar.activation(ui[:, :], zt[0][:, :], SIG,
                                         bias=bv[:, 0:1])
                    uf = wpool.tile([128, NT], BF, tag="uf")
                    nc.scalar.activation(uf[:, :], zt[1][:, :], SIG,
                                         bias=bv[:, 1:2])
                    g_all = wpool.tile([128, NT], BF, tag="gall")
                    nc.scalar.activation(g_all[:, :], zt[2][:, :], TANH,
                                         bias=bv[:, 2:3])
                    uo = wpool.tile([128, NT], BF, tag="uo")
                    nc.scalar.activation(uo[:, :], zt[3][:, :], SIG,
                                         bias=bv[:, 3:4])
                    ui, uf, uo = ui[:, :], uf[:, :], uo[:, :]

                    if tau == 0:
                        # c = i * g  (h-slots and previous c are zero)
                        nc.vector.tensor_mul(c_all[:, :], ui, g_all[:, :])
                    else:
                        t1 = wpool.tile([128, NT], BF, tag="t1")
                        nc.vector.tensor_mul(t1[:, :], ui, g_all[:, :])
                        t2 = wpool.tile([128, NT], FP32, tag="t2")
                        nc.vector.tensor_mul(t2[:, :], uf, c_all[:, :])
                        nc.vector.tensor_add(c_all[:, :], t1[:, :], t2[:, :])

                    tc_t = wpool.tile([128, NT], BF, tag="tc")
                    nc.scalar.activation(tc_t[:, :], c_all[:, :], TANH)
                    h_new = wpool.tile([128, NT], BF, tag="hn")
                    nc.vector.tensor_mul(h_new[:, :], uo, tc_t[:, :])

                    for s in range(4):
                        g_t = gs[s // 2]
                        bw = s % 2
                        src = h_new[32 * s:32 * s + 32, :]
                        if tau == T - 1:
                            col0 = (2 * pair + s // 2) * NT
                            dst = state[32 * bw:32 * bw + 32,
                                        col0:col0 + NT]
                        else:
                            nblk = (T - 2 - tau) if bw else (tau + 1)
                            h0 = HBW0 if bw else HFW0
                            dst = g_t[h0:h0 + 32, 0,
                                      nblk * NT:(nblk + 1) * NT]
                        nc.vector.tensor_copy(dst, src)

          # ---- dense head: out[c, b] = tanh(sum_p Wd_p.T @ state_p + bd)
          if True:
            wd = cpool.tile([64, 4096], BF)
            nc.sync.dma_start(out=wd[:, :], in_=wd_d.ap())
            bd = cpool.tile([64, 1], FP32)
            nc.sync.dma_start(out=bd[:, :], in_=bd_d.ap())

            with tc.tile_pool(name="head", bufs=1, space="PSUM") as hpool:
                st_r = state[:, :].rearrange("p (b q) -> p q b", q=P)
                hp = hpool.tile([64, nbatch], FP32)
                for p in range(P):
                    nc.tensor.matmul(hp[:, :], wd[:, 64 * p:64 * p + 64],
                                     st_r[:, p:p + 1, :],
                                     start=(p == 0), stop=(p == P - 1))
                out_sb = cpool.tile([64, nbatch], FP32)
                nc.scalar.activation(out_sb[:, :], hp[:, :], TANH,
                                     bias=bd[:, :])
                nc.sync.dma_start(out=out_d.ap(), in_=out_sb[:, :])
                nc.sync.dma_start(out=state_d.ap(), in_=state[:, :])

    nc.compile()
    return nc


# ---------------------------------------------------------------------------
# host-side packing
# ---------------------------------------------------------------------------

def pack_table(embed_table):
    tbl = np.zeros((VOCP, 128), np.float32)
    tbl[:VOC, EMB0:EMB0 + 32] = _f32(embed_table)
    tbl[:VOC, EMB1:EMB1 + 32] = _f32(embed_table)
    return tbl.astype(BF16)


def pack_idx(x_core, nseq=NSEQ):
    """x_core: [nseq, T] int32 -> wrapped int16 [128, ntiles * T*NT/16]."""
    ntiles = nseq // NT
    cols = []
    for j in range(ntiles):
        u = x_core[j * NT:(j + 1) * NT, :].T.reshape(-1)     # t-major [T*NT]
        w = u.reshape(-1, 16).T                               # [16, T*NT/16]
        cols.append(np.tile(w, (8, 1)))
    return np.concatenate(cols, axis=1).astype(np.int16)


def pack_weights(Wk, Wr, b):
    Wk, Wr, b = _f32(Wk), _f32(Wr), _f32(b)
    wf = np.concatenate([Wr, Wk], 0)                          # [64, 128]
    wb = np.concatenate([np.zeros((64, 128), np.float32), Wk, Wr], 0)
    bv = np.tile(b.reshape(4, 32), (1, 4)).reshape(4, 128).T  # [128, 4]
    return wf.astype(BF16), wb.astype(BF16), np.ascontiguousarray(bv, np.float32)


def pack_wd(Wd):
    w = _f32(Wd).reshape(P, 64, 64).transpose(1, 0, 2).reshape(64, 4096)
    return w.astype(BF16)


# ---------------------------------------------------------------------------
# host reference bits for the zero-token fixup
# ---------------------------------------------------------------------------

def _np_lstm_last_h(emb, mask, Wk, Wr, b):
    n = emb.shape[0]
    h = np.zeros((n, H), np.float32)
    c = np.zeros((n, H), np.float32)
    for t in range(emb.shape[1]):
        z = emb[:, t, :] @ Wk + h @ Wr + b
        i = 1.0 / (1.0 + np.exp(-z[:, 0:32]))
        f = 1.0 / (1.0 + np.exp(-z[:, 32:64]))
        g = np.tanh(z[:, 64:96])
        o = 1.0 / (1.0 + np.exp(-z[:, 96:128]))
        c_new = f * c + i * g
        h_new = o * np.tanh(c_new)
        m = mask[:, t][:, None]
        h = np.where(m, h_new, h)
        c = np.where(m, c_new, c)
    return h


def _host_fixup(out, state_all, x_flat, embed_table, Wk, Wr, b, Wd, bd):
    """Recompute rows whose sequences contain a zero token."""
    mask = x_flat != 0
    bad_seq = np.nonzero(~mask.all(axis=1))[0]
    if bad_seq.size == 0:
        return out
    emb = _f32(embed_table)[x_flat[bad_seq]]
    h_fw = _np_lstm_last_h(emb, mask[bad_seq], _f32(Wk), _f32(Wr), _f32(b))
    h_bw = _np_lstm_last_h(emb[:, ::-1, :], mask[bad_seq][:, ::-1],
                           _f32(Wk), _f32(Wr), _f32(b))
    state_all = state_all.copy()
    state_all[bad_seq] = np.concatenate([h_fw, h_bw], axis=1)
    bad_rows = np.unique(bad_seq // P)
    st = state_all[bad_rows[:, None] * P + np.arange(P)[None, :]]
    st = st.reshape(bad_rows.size, P * 64)
    out[bad_rows] = np.tanh(st @ _f32(Wd) + _f32(bd))
    return out


# ---------------------------------------------------------------------------
# entry point
# ---------------------------------------------------------------------------

_NC_CACHE = {}


def _get_nc(mode="full"):
    key = "nc" + mode
    if key not in _NC_CACHE:
        _NC_CACHE[key] = build_kernel(mode=mode)
    return _NC_CACHE[key]


def run_device(inputs, trace=False):
    x = np.asarray(inputs["x"])
    table = pack_table(inputs["embed_table"])
    wf, wb, bv = pack_weights(inputs["Wk"], inputs["Wr"], inputs["b"])
    wd = pack_wd(inputs["Wd"])
    bd = _f32(inputs["bd"]).reshape(64, 1)

    x_flat = x.reshape(B * P, L)
    in_maps = []
    for k in range(N_CORES):
        x_core = x_flat[k * NSEQ:(k + 1) * NSEQ]
        in_maps.append({
            "table": table,
            "idx": pack_idx(x_core),
            "wf": wf,
            "wb": wb,
            "bv": bv,
            "wd": wd,
            "bd": bd,
        })

    nc = _get_nc()
    res = bass_utils.run_bass_kernel_spmd(
        nc, in_maps, core_ids=list(range(N_CORES)), trace=trace)

    out = np.empty((B, 64), np.float32)
    state_all = np.empty((B * P, 64), np.float32)
    for k in range(N_CORES):
        out[k * B_LOC:(k + 1) * B_LOC] = res.results[k]["out"].T
        state_all[k * NSEQ:(k + 1) * NSEQ] = \
            _f32(res.results[k]["state"]).T
    return out, state_all, res


def kernel(x, embed_table, Wk, Wr, b, Wd, bd):
    inputs = dict(x=x, embed_table=embed_table, Wk=Wk, Wr=Wr, b=b,
                  Wd=Wd, bd=bd)
    out, state_all, _ = run_device(inputs)
    out = _host_fixup(out, state_all, np.asarray(x).reshape(B * P, L),
                      embed_table, Wk, Wr, b, Wd, bd)
    return out



# revision 2
# speedup vs baseline: 1080.2294x; 1080.2294x over previous
"""CardEncoder Trainium2 kernel — v4.

Model per sequence (L=16 tokens): embed(32) -> biLSTM(32) -> concat final
states -> dense (4096 -> 64) -> tanh.  Data parallel over 8 cores.

v4 design:
  * Embedding table resident in SBUF (2.6MB bf16, partition-major); gather is
    SBUF->SBUF dma_gather(transpose=True).
  * Block-diagonal weights [128, 64] per gate compute fw+bw streams of one
    tile in ONE matmul (8 matmuls/step instead of 16).  Requires the bw
    embedding copy time-reversed in SBUF (16 DVE copies per tile) so fw and
    bw read the same column block at each step.
  * Two tile-pairs (8 streams) interleaved per super-iteration so the
    per-step serial chain (PE -> ACT -> DVE -> PE) pipelines across
    independent LSTM chains.
  * G tiles double-buffered (gathers for super s+1 prefetch during s);
    idx, state, and the dense head staged per super-iteration.
  * mask_zero handling: zero tokens occur w.p. 1e-4; host recomputes affected
    rows exactly and patches the output.
"""

import os
import numpy as np
import ml_dtypes

os.environ.setdefault("JAX_PLATFORMS", "cpu")

import concourse.bass as bass
import concourse.bacc as bacc
import concourse.mybir as mybir
import concourse.tile as tile
from concourse import bass_utils

BF16 = ml_dtypes.bfloat16

B, P, L = 2048, 64, 16
H = 32
VOC = 10000
VOCP = 10112                # 79 * 128
N_CORES = 8
B_LOC = B // N_CORES
NSEQ = B_LOC * P            # 16384 sequences per core
NT = 512
T = L

# G tile partition layout:
#   [ h_fw 0:32 | emb 32:64 | emb_rev 64:96 | h_bw 96:128 ]
# fw fmap rows 0:64 = [h_fw | emb]; bw fmap rows 64:128 = [emb_rev | h_bw].
# Both read column block tau at step tau (emb_rev is time-reversed on chip).
HFW0 = 0
EMB0 = 32
EMBR0 = 64
HBW0 = 96


def _f32(x):
    return np.asarray(x, np.float32)


# ---------------------------------------------------------------------------
# device kernel
# ---------------------------------------------------------------------------

def build_kernel(nseq=NSEQ, mode="full", gather="sbuf", reps=1, ilv=2):
    ntiles = nseq // NT
    npairs = ntiles // 2
    nsuper = npairs // ilv
    assert nsuper * ilv == npairs
    nbatch = nseq // P
    bat_sup = 2 * ilv * NT // P          # batches per super-iteration

    nc = bacc.Bacc("TRN2", target_bir_lowering=False, debug=False,
                   enable_asserts=False, num_devices=N_CORES)

    if gather == "sbuf":
        table_d = nc.dram_tensor("table", [128, VOCP], mybir.dt.bfloat16,
                                 kind="ExternalInput")
    else:
        table_d = nc.dram_tensor("table", [VOCP, 128], mybir.dt.bfloat16,
                                 kind="ExternalInput")
    idx_d = nc.dram_tensor("idx", [128, ntiles * NT * T // 16], mybir.dt.int16,
                           kind="ExternalInput")
    wbd_d = nc.dram_tensor("wbd", [128, 256], mybir.dt.bfloat16,
                           kind="ExternalInput")
    bv_d = nc.dram_tensor("bv", [128, 4], mybir.dt.float32,
                          kind="ExternalInput")
    wd_d = nc.dram_tensor("wd", [64, 4096], mybir.dt.bfloat16,
                          kind="ExternalInput")
    bd_d = nc.dram_tensor("bd", [64, 1], mybir.dt.float32,
                          kind="ExternalInput")
    out_d = nc.dram_tensor("out", [64, nbatch], mybir.dt.float32,
                           kind="ExternalOutput")
    state_d = nc.dram_tensor("state", [64, nseq], mybir.dt.bfloat16,
                             kind="ExternalOutput")

    FP32 = mybir.dt.float32
    BF = mybir.dt.bfloat16
    SIG = mybir.ActivationFunctionType.Sigmoid
    TANH = mybir.ActivationFunctionType.Tanh
    IDXW = NT * T // 16                  # idx cols per tile
    SUPW = 2 * ilv * IDXW                # idx cols per super-iteration
    SUPS = 2 * ilv * NT                  # seqs per super-iteration

    with tile.TileContext(nc) as tc:
        with tc.tile_pool(name="const", bufs=1) as cpool:
            wbd = cpool.tile([128, 256], BF)
            nc.sync.dma_start(out=wbd[:, :], in_=wbd_d.ap())
            bv = cpool.tile([128, 4], FP32)
            nc.sync.dma_start(out=bv[:, :], in_=bv_d.ap())
            wd = cpool.tile([64, 4096], BF)
            nc.sync.dma_start(out=wd[:, :], in_=wd_d.ap())
            bd = cpool.tile([64, 1], FP32)
            nc.sync.dma_start(out=bd[:, :], in_=bd_d.ap())
            if gather == "sbuf":
                tbl_sb = cpool.tile([128, VOCP], BF)
                nc.sync.dma_start(out=tbl_sb[:, :], in_=table_d.ap())

            with tc.tile_pool(name="gbuf", bufs=2) as gpool, \
                 tc.tile_pool(name="stage", bufs=2) as spool, \
                 tc.tile_pool(name="work", bufs=1) as wpool, \
                 tc.tile_pool(name="zps", bufs=2, space="PSUM") as zpool:

                def gather_tile(j, tag, idx_sb, jloc):
                    g = gpool.tile([128, 1, T * NT], BF, tag=tag,
                                   name=f"g_{j}")
                    if mode == "compute":
                        return g
                    idxs = idx_sb[:, jloc * IDXW:(jloc + 1) * IDXW]
                    if gather == "sbuf":
                        nc.gpsimd.dma_gather(
                            out_ap=g[:, :, :], in_ap=tbl_sb[:, :],
                            idxs_ap=idxs,
                            num_idxs=T * NT, num_idxs_reg=T * NT,
                            elem_size=128, transpose=True,
                            single_packet=False,
                            sbuf_tokens_per_rank=128,
                            sbuf_free_dim_per_rank=256,
                            sbuf_free_dim_pad_per_rank=0,
                            sbuf_byte_offset=0)
                    else:
                        nc.gpsimd.dma_gather(
                            out_ap=g[:, :, :], in_ap=table_d.ap(),
                            idxs_ap=idxs,
                            num_idxs=T * NT, num_idxs_reg=T * NT,
                            elem_size=128, transpose=True,
                            single_packet=False)
                    # time-reverse the emb copy for the bw direction
                    for t in range(T):
                        nc.vector.tensor_copy(
                            g[EMBR0:EMBR0 + 32, 0,
                              (T - 1 - t) * NT:(T - t) * NT],
                            g[EMB0:EMB0 + 32, 0, t * NT:(t + 1) * NT])
                    return g

                def emit_step(gs, c_all, st_sup, tau, pair, k2, sup):
                    zt = [zpool.tile([128, NT], FP32, tag=f"z{gi}",
                                     name=f"z{gi}_{pair}_{tau}")
                          for gi in range(4)]
                    for ti in range(2):
                        g_t = gs[ti]
                        rhs = g_t[0:128, 0, tau * NT:(tau + 1) * NT]
                        for gi in range(4):   # gate order i,f,g,o
                            nc.tensor.matmul(
                                zt[gi][64 * ti:64 * ti + 64, :],
                                wbd[:, 64 * gi:64 * gi + 64], rhs,
                                start=True, stop=True,
                                tile_position=(0, 64 * ti))

                    ui = wpool.tile([128, NT], BF, tag=f"ui{k2}")
                    nc.scalar.activation(ui[:, :], zt[0][:, :], SIG,
                                         bias=bv[:, 0:1])
                    uf = wpool.tile([128, NT], BF, tag=f"uf{k2}")
                    nc.scalar.activation(uf[:, :], zt[1][:, :], SIG,
                                         bias=bv[:, 1:2])
                    g_all = wpool.tile([128, NT], BF, tag=f"ug{k2}")
                    nc.scalar.activation(g_all[:, :], zt[2][:, :], TANH,
                                         bias=bv[:, 2:3])
                    uo = wpool.tile([128, NT], BF, tag=f"uo{k2}")
                    nc.scalar.activation(uo[:, :], zt[3][:, :], SIG,
                                         bias=bv[:, 3:4])

                    if tau == 0:
                        nc.vector.tensor_mul(c_all[:, :], ui[:, :],
                                             g_all[:, :])
                    else:
                        t1 = wpool.tile([128, NT], BF, tag=f"t1{k2}")
                        nc.vector.tensor_mul(t1[:, :], ui[:, :], g_all[:, :])
                        t2 = wpool.tile([128, NT], FP32, tag=f"t2{k2}")
                        nc.vector.tensor_mul(t2[:, :], uf[:, :], c_all[:, :])
                        nc.vector.tensor_add(c_all[:, :], t1[:, :], t2[:, :])

                    tc_t = wpool.tile([128, NT], BF, tag=f"tc{k2}")
                    nc.scalar.activation(tc_t[:, :], c_all[:, :], TANH)
                    h_new = wpool.tile([128, NT], BF, tag=f"hn{k2}")
                    nc.vector.tensor_mul(h_new[:, :], uo[:, :], tc_t[:, :])

                    # streams: [0:32]=fw_a [32:64]=bw_a [64:96]=fw_b [96:128]=bw_b
                    for s in range(4):
                        g_t = gs[s // 2]
                        bw = s % 2
                        src = h_new[32 * s:32 * s + 32, :]
                        if tau == T - 1:
                            col0 = (2 * k2 + s // 2) * NT
                            dst = st_sup[32 * bw:32 * bw + 32,
                                         col0:col0 + NT]
                        else:
                            h0 = HBW0 if bw else HFW0
                            dst = g_t[h0:h0 + 32, 0,
                                      (tau + 1) * NT:(tau + 2) * NT]
                        nc.vector.tensor_copy(dst, src)

                rep_ctx = tc.For_i(0, reps, 1) if reps > 1 else None
                if rep_ctx is not None:
                    rep_ctx.__enter__()
                for sup in range(nsuper):
                    if mode == "empty":
                        break
                    idx_sb = spool.tile([128, SUPW], mybir.dt.int16,
                                        tag="idx", name=f"idx_{sup}")
                    nc.sync.dma_start(
                        out=idx_sb[:, :],
                        in_=idx_d.ap()[:, sup * SUPW:(sup + 1) * SUPW])
                    st_sup = spool.tile([64, SUPS], BF, tag="st",
                                        name=f"st_{sup}")
                    gs_k = []
                    for k2 in range(ilv):
                        pair = sup * ilv + k2
                        ga = gather_tile(2 * pair, f"g{2 * k2}", idx_sb,
                                         2 * k2)
                        gb = gather_tile(2 * pair + 1, f"g{2 * k2 + 1}",
                                         idx_sb, 2 * k2 + 1)
                        gs_k.append([ga, gb])
                    if mode == "gather":
                        for k2 in range(ilv):
                            for gi2, g_t in enumerate(gs_k[k2]):
                                col0 = (2 * k2 + gi2) * NT
                                nc.vector.tensor_copy(
                                    st_sup[0:32, col0:col0 + NT],
                                    g_t[32:64, 0, (T - 1) * NT:T * NT])
                        nc.sync.dma_start(
                            out=state_d.ap()[:, sup * SUPS:(sup + 1) * SUPS],
                            in_=st_sup[:, :])
                        continue
                    c_k = [wpool.tile([128, NT], FP32, tag=f"c{k2}", bufs=2,
                                      name=f"c_{sup}_{k2}")
                           for k2 in range(ilv)]
                    for tau in range(T):
                        for k2 in range(ilv):
                            emit_step(gs_k[k2], c_k[k2], st_sup, tau,
                                      sup * ilv + k2, k2, sup)

                    nc.sync.dma_start(
                        out=state_d.ap()[:, sup * SUPS:(sup + 1) * SUPS],
                        in_=st_sup[:, :])

                    # dense head for this super's batches
                    st_r = st_sup[:, :].rearrange("p (b q) -> p q b", q=P)
                    hp = zpool.tile([128, NT], FP32, tag="z0",
                                    name=f"hp_{sup}")
                    for p in range(P):
                        nc.tensor.matmul(hp[0:64, 0:bat_sup],
                                         wd[:, 64 * p:64 * p + 64],
                                         st_r[:, p:p + 1, :],
                                         start=(p == 0), stop=(p == P - 1))
                    out_sb = spool.tile([64, bat_sup], FP32, tag="out",
                                        name=f"out_{sup}")
                    nc.scalar.activation(out_sb[:, :], hp[0:64, 0:bat_sup],
                                         TANH, bias=bd[:, :])
                    nc.sync.dma_start(
                        out=out_d.ap()[:, sup * bat_sup:(sup + 1) * bat_sup],
                        in_=out_sb[:, :])

                if rep_ctx is not None:
                    rep_ctx.__exit__(None, None, None)

    nc.compile()
    return nc


# ---------------------------------------------------------------------------
# host-side packing
# ---------------------------------------------------------------------------

def pack_table(embed_table):
    tbl = np.zeros((VOCP, 128), np.float32)
    tbl[:VOC, EMB0:EMB0 + 32] = _f32(embed_table)
    tbl[:VOC, EMBR0:EMBR0 + 32] = _f32(embed_table)
    return tbl.astype(BF16)


def pack_table_pm(embed_table):
    tbl = pack_table(embed_table)
    return np.ascontiguousarray(
        tbl.reshape(VOCP // 128, 128, 128).transpose(1, 0, 2)
        .reshape(128, VOCP))


def pack_idx(x_core, nseq=NSEQ):
    ntiles = nseq // NT
    cols = []
    for j in range(ntiles):
        u = x_core[j * NT:(j + 1) * NT, :].T.reshape(-1)
        w = u.reshape(-1, 16).T
        cols.append(np.tile(w, (8, 1)))
    return np.concatenate(cols, axis=1).astype(np.int16)


def pack_weights(Wk, Wr, b):
    """Block-diagonal per-gate weights [128, 4*64] and bias [128, 4]."""
    Wk, Wr, b = _f32(Wk), _f32(Wr), _f32(b)
    wf = np.concatenate([Wr, Wk], 0)          # [64, 128] fw: fmap [h|emb]
    wbw = np.concatenate([Wk, Wr], 0)         # [64, 128] bw: fmap [emb|h]
    wbd = np.zeros((128, 256), np.float32)
    for gi in range(4):
        wbd[0:64, 64 * gi:64 * gi + 32] = wf[:, 32 * gi:32 * gi + 32]
        wbd[64:128, 64 * gi + 32:64 * gi + 64] = wbw[:, 32 * gi:32 * gi + 32]
    bv = np.tile(b.reshape(4, 32), (1, 4)).reshape(4, 128).T
    return wbd.astype(BF16), np.ascontiguousarray(bv, np.float32)


def pack_wd(Wd):
    w = _f32(Wd).reshape(P, 64, 64).transpose(1, 0, 2).reshape(64, 4096)
    return w.astype(BF16)


# ---------------------------------------------------------------------------
# host reference bits for the zero-token fixup
# ---------------------------------------------------------------------------

def _np_lstm_last_h(emb, mask, Wk, Wr, b):
    n = emb.shape[0]
    h = np.zeros((n, H), np.float32)
    c = np.zeros((n, H), np.float32)
    for t in range(emb.shape[1]):
        z = emb[:, t, :] @ Wk + h @ Wr + b
        i = 1.0 / (1.0 + np.exp(-z[:, 0:32]))
        f = 1.0 / (1.0 + np.exp(-z[:, 32:64]))
        g = np.tanh(z[:, 64:96])
        o = 1.0 / (1.0 + np.exp(-z[:, 96:128]))
        c_new = f * c + i * g
        h_new = o * np.tanh(c_new)
        m = mask[:, t][:, None]
        h = np.where(m, h_new, h)
        c = np.where(m, c_new, c)
    return h


def _host_fixup(out, state_all, x_flat, embed_table, Wk, Wr, b, Wd, bd):
    mask = x_flat != 0
    bad_seq = np.nonzero(~mask.all(axis=1))[0]
    if bad_seq.size == 0:
        return out
    emb = _f32(embed_table)[x_flat[bad_seq]]
    h_fw = _np_lstm_last_h(emb, mask[bad_seq], _f32(Wk), _f32(Wr), _f32(b))
    h_bw = _np_lstm_last_h(emb[:, ::-1, :], mask[bad_seq][:, ::-1],
                           _f32(Wk), _f32(Wr), _f32(b))
    state_all = state_all.copy()
    state_all[bad_seq] = np.concatenate([h_fw, h_bw], axis=1)
    bad_rows = np.unique(bad_seq // P)
    st = state_all[bad_rows[:, None] * P + np.arange(P)[None, :]]
    st = st.reshape(bad_rows.size, P * 64)
    out[bad_rows] = np.tanh(st @ _f32(Wd) + _f32(bd))
    return out


# ---------------------------------------------------------------------------
# entry point
# ---------------------------------------------------------------------------

_NC_CACHE = {}


def _get_nc(mode="full", gather="sbuf", reps=1, ilv=2):
    key = f"nc{mode}{gather}{reps}{ilv}"
    if key not in _NC_CACHE:
        _NC_CACHE[key] = build_kernel(mode=mode, gather=gather, reps=reps,
                                      ilv=ilv)
    return _NC_CACHE[key]


def run_device(inputs, trace=False, gather="sbuf", reps=1, ilv=2):
    x = np.asarray(inputs["x"])
    if gather == "sbuf":
        table = pack_table_pm(inputs["embed_table"])
    else:
        table = pack_table(inputs["embed_table"])
    wbd, bv = pack_weights(inputs["Wk"], inputs["Wr"], inputs["b"])
    wd = pack_wd(inputs["Wd"])
    bd = _f32(inputs["bd"]).reshape(64, 1)

    x_flat = x.reshape(B * P, L)
    in_maps = []
    for k in range(N_CORES):
        x_core = x_flat[k * NSEQ:(k + 1) * NSEQ]
        in_maps.append({
            "table": table,
            "idx": pack_idx(x_core),
            "wbd": wbd,
            "bv": bv,
            "wd": wd,
            "bd": bd,
        })

    nc = _get_nc(gather=gather, reps=reps, ilv=ilv)
    res = bass_utils.run_bass_kernel_spmd(
        nc, in_maps, core_ids=list(range(N_CORES)), trace=trace)

    out = np.empty((B, 64), np.float32)
    state_all = np.empty((B * P, 64), np.float32)
    for k in range(N_CORES):
        out[k * B_LOC:(k + 1) * B_LOC] = res.results[k]["out"].T
        state_all[k * NSEQ:(k + 1) * NSEQ] = \
            _f32(res.results[k]["state"]).T
    return out, state_all, res


def kernel(x, embed_table, Wk, Wr, b, Wd, bd):
    inputs = dict(x=x, embed_table=embed_table, Wk=Wk, Wr=Wr, b=b,
                  Wd=Wd, bd=bd)
    out, state_all, _ = run_device(inputs)
    out = _host_fixup(out, state_all, np.asarray(x).reshape(B * P, L),
                      embed_table, Wk, Wr, b, Wd, bd)
    return out
